# revision 25
# baseline (speedup 1.0000x reference)
"""BackflowMLP Trainium2 kernel.

Strategy: the dense MLP (all the FLOPs, dominated by the 512x512 @
512x65536 output matmul) runs on 8 NeuronCores with 2D sharding:
4-way tensor-parallel over the 65536 output dim x 2-way data-parallel
over the batch. TP keeps the PE's moving dim at 256 columns (full
streaming rate); the DP split halves the replicated trunk-MLP work.
Activations are feature-major ([feature_partition, batch_free]) so
biases are per-partition scalars and gelu runs straight off PSUM.
Matmuls run in bf16 (1 cycle/row on TRN2 PE vs 4 for fp32) with fp32
PSUM accumulation; residual adds in fp32. The tiny complex
gather/logdet/logsumexp tail runs on host.
"""

import numpy as np
import ml_dtypes

N_ORB, N_A, N_B, N_DETS = 64, 32, 32, 16
K = 32
H0 = H1 = 512
IN_DIM = 128
OUT_DIM = N_DETS * N_ORB * K            # 32768
OUT2 = 2 * OUT_DIM                      # 65536
B = 512
NCORES = 8
TP, DP = 8, 1
BSH = B // DP                           # 256 batch rows per core
OSH = OUT2 // TP                        # 16384 output features per core
OT = OSH // 128                         # 128 output tiles per core

_CACHE = {}


def _build_nc():
    import concourse.mybir as mybir
    import concourse.tile as tile
    from concourse import bacc

    fp32 = mybir.dt.float32
    bf16 = mybir.dt.bfloat16
    GELU = mybir.ActivationFunctionType.Gelu_apprx_tanh
    INV_SQRT2 = float(1.0 / np.sqrt(2.0))

    nc = bacc.Bacc(
        "TRN2", target_bir_lowering=False, debug=False, num_devices=NCORES
    )

    xT = nc.declare_dram_parameter("xT", [IN_DIM, BSH], bf16, isOutput=False)
    # Wc = Ws0 + gelu(1)*W0, merged on host: s is exactly {0,1}, so
    # gelu(s) = gelu(1)*s and block 0 is a single matmul.
    Wc = nc.declare_dram_parameter("Wc", [IN_DIM, H0], bf16, isOutput=False)
    W1 = nc.declare_dram_parameter("W1", [H0, H1], bf16, isOutput=False)
    Wout = nc.declare_dram_parameter("Wout", [H1, OSH], bf16, isOutput=False)
    bias0 = nc.declare_dram_parameter("bias0", [128, 4], fp32, isOutput=False)
    bias1 = nc.declare_dram_parameter("bias1", [128, 4], fp32, isOutput=False)
    boutp = nc.declare_dram_parameter("boutp", [128, OT], fp32, isOutput=False)
    yT = nc.declare_dram_parameter("yT", [OSH, BSH], bf16, isOutput=True)

    mult = mybir.AluOpType.mult
    add = mybir.AluOpType.add

    with tile.TileContext(nc) as tc:
        with (
            tc.tile_pool(name="wp", bufs=1) as wp,
            tc.tile_pool(name="ap_", bufs=1) as ap_,
            tc.tile_pool(name="op", bufs=8) as op,
            tc.tile_pool(name="pps", bufs=3, space="PSUM") as pps,
        ):
            # ---- persistent loads ----
            xt = wp.tile([128, BSH], bf16, tag="xt")
            nc.sync.dma_start(xt[:], xT[:])
            wc = wp.tile([128, H0], bf16, tag="wc")
            nc.sync.dma_start(wc[:], Wc[:])
            w1 = wp.tile([128, 4 * H1], bf16, tag="w1")
            for kc in range(4):
                nc.sync.dma_start(
                    w1[:, kc * H1:(kc + 1) * H1], W1[kc * 128:(kc + 1) * 128, :]
                )
            b0t = wp.tile([128, 4], fp32, tag="b0t")
            nc.sync.dma_start(b0t[:], bias0[:])
            b1t = wp.tile([128, 4], fp32, tag="b1t")
            nc.sync.dma_start(b1t[:], bias1[:])
            bot = wp.tile([128, OT], fp32, tag="bot")
            nc.sync.dma_start(bot[:], boutp[:])
            # Wout: 4 K-chunks of [128, OSH]; DMA split into 8 pieces along
            # the feature dim so early output tiles can start sooner.
            wout = wp.tile([128, 4 * OSH], bf16, tag="wout")
            NPIECE = 8
            PIECE = OSH // NPIECE
            for piece in range(NPIECE):
                for kc in range(4):
                    nc.sync.dma_start(
                        wout[:, kc * OSH + piece * PIECE: kc * OSH + (piece + 1) * PIECE],
                        Wout[kc * 128:(kc + 1) * 128, piece * PIECE:(piece + 1) * PIECE],
                    )

            # ---- trunk: residual block 0 (skip + gelu branch, merged) ----
            x1 = []
            g1 = []
            for mt in range(4):
                r_ps = pps.tile([128, BSH], fp32, tag="ps_a")
                nc.tensor.matmul(
                    r_ps[:], wc[:, mt * 128:(mt + 1) * 128], xt[:],
                    start=True, stop=True,
                )
                x1t = ap_.tile([128, BSH], fp32, tag=f"x1_{mt}")
                nc.vector.tensor_scalar_add(x1t[:], r_ps[:], b0t[:, mt:mt + 1])
                g1t = ap_.tile([128, BSH], bf16, tag=f"g1_{mt}")
                nc.scalar.activation(g1t[:], x1t[:], GELU)
                x1.append(x1t)
                g1.append(g1t)

            # ---- trunk: residual block 1 (identity skip) ----
            g2 = []
            for mt in range(4):
                h1_ps = pps.tile([128, BSH], fp32, tag="ps_a")
                for kc in range(4):
                    nc.tensor.matmul(
                        h1_ps[:],
                        w1[:, kc * H1 + mt * 128: kc * H1 + (mt + 1) * 128],
                        g1[kc][:],
                        start=(kc == 0), stop=(kc == 3),
                    )
                x2t = ap_.tile([128, BSH], fp32, tag=f"x2_{mt}")
                nc.vector.tensor_add(x2t[:], h1_ps[:], x1[mt][:])
                nc.vector.tensor_scalar_add(x2t[:], x2t[:], b1t[:, mt:mt + 1])
                g2t = ap_.tile([128, BSH], bf16, tag=f"g2_{mt}")
                nc.scalar.activation(g2t[:], x2t[:], GELU)
                g2.append(g2t)

            # ---- big output matmul, OT tiles of [128 feat, BSH batch] ----
            for ot in range(OT):
                y_ps = pps.tile([128, BSH], fp32, tag="ps_y")
                for kc in range(4):
                    nc.tensor.matmul(
                        y_ps[:],
                        wout[:, kc * OSH + ot * 128: kc * OSH + (ot + 1) * 128],
                        g2[kc][:],
                        start=(kc == 0), stop=(kc == 3),
                    )
                # split the PSUM drain across engines so neither paces the
                # PE: even tiles ACT Copy (PSUM read + scale) then a fast
                # bf16 DVE bias-add; odd tiles the all-DVE fused epilogue
                yt = op.tile([128, BSH], bf16, tag="yt")
                if ot % 2 == 0:
                    yt2 = op.tile([128, BSH], bf16, tag="yt2")
                    nc.scalar.activation(
                        yt2[:], y_ps[:], mybir.ActivationFunctionType.Copy,
                        scale=INV_SQRT2,
                    )
                    nc.vector.tensor_scalar_add(yt[:], yt2[:], bot[:, ot:ot + 1])
                else:
                    nc.vector.tensor_scalar(
                        yt[:], y_ps[:], INV_SQRT2, bot[:, ot:ot + 1],
                        op0=mult, op1=add,
                    )
                nc.sync.dma_start(yT[ot * 128:(ot + 1) * 128, :], yt[:])
    nc.compile()
    return nc


def _get_nc():
    if "nc" not in _CACHE:
        _CACHE["nc"] = _build_nc()
    return _CACHE["nc"]


def kernel(**inputs):
    import os
    import time
    os.environ["BASS_NEVER_TRACE"] = "1"   # NTFF hook module absent in this build
    from concourse import bass_utils

    s = np.asarray(inputs["s"])
    W0 = np.asarray(inputs["W0"], np.float32)
    b0 = np.asarray(inputs["b0"], np.float32)
    Ws0 = np.asarray(inputs["Ws0"], np.float32)
    bs0 = np.asarray(inputs["bs0"], np.float32)
    W1 = np.asarray(inputs["W1"], np.float32)
    b1 = np.asarray(inputs["b1"], np.float32)
    Wout = np.asarray(inputs["Wout"], np.float32)
    bout = np.asarray(inputs["bout"], np.float32)
    M = np.asarray(inputs["M"])
    log_c = np.asarray(inputs["log_c"])

    bf = ml_dtypes.bfloat16
    xT_full = np.ascontiguousarray(s.astype(np.float32).T).astype(bf)   # [128, 512]
    bias0 = np.ascontiguousarray((b0 + bs0).reshape(4, 128).T).astype(np.float32)
    bias1 = np.ascontiguousarray(b1.reshape(4, 128).T).astype(np.float32)
    # s is {0,1} exactly, so gelu(s) = gelu(1)*s: fold the gelu branch of
    # block 0 into the skip projection (tanh-approx gelu at x=1, fp64).
    g1c = 0.5 * (1.0 + np.tanh(np.sqrt(2.0 / np.pi) * (1.0 + 0.044715)))
    Wcb = (Ws0.astype(np.float64) + g1c * W0.astype(np.float64)).astype(bf)
    W1b = W1.astype(bf)

    wsh = []
    bsh = []
    for tp in range(TP):
        wsh.append(np.ascontiguousarray(Wout[:, tp * OSH:(tp + 1) * OSH]).astype(bf))
        bo = bout[tp * OSH:(tp + 1) * OSH] / np.sqrt(2.0)
        bsh.append(np.ascontiguousarray(bo.reshape(OT, 128).T).astype(np.float32))

    in_maps = []
    for i in range(NCORES):
        dp, tp = divmod(i, TP)
        in_maps.append({
            "xT": np.ascontiguousarray(xT_full[:, dp * BSH:(dp + 1) * BSH]),
            "Wc": Wcb,
            "W1": W1b,
            "Wout": wsh[tp],
            "bias0": bias0,
            "bias1": bias1,
            "boutp": bsh[tp],
        })

    nc = _get_nc()
    t0 = time.perf_counter()
    res = bass_utils.run_bass_kernel_spmd(nc, in_maps, core_ids=list(range(NCORES)))
    _CACHE["last_exec_ns"] = res.exec_time_ns
    _CACHE["last_wall_ns"] = (time.perf_counter() - t0) * 1e9

    y = np.empty((B, OUT2), np.float32)
    for i in range(NCORES):
        dp, tp = divmod(i, TP)
        y[dp * BSH:(dp + 1) * BSH, tp * OSH:(tp + 1) * OSH] = (
            res.results[i]["yT"].astype(np.float32).T
        )

    # ---- host tail: complex assembly, gather, logdet, logsumexp ----
    re, im = y[:, :OUT_DIM], y[:, OUT_DIM:]
    delta = (re + 1j * im).astype(np.complex64).reshape(B, N_DETS, N_ORB, K)
    M_eff = M[None].astype(np.complex64) + delta

    rows_a = np.argsort(1 - s[:, :N_ORB], axis=-1, kind="stable")[:, :N_A]
    rows_b = np.argsort(1 - s[:, N_ORB:], axis=-1, kind="stable")[:, :N_B]
    ia = np.broadcast_to(rows_a[:, None, :, None], (B, N_DETS, N_A, K))
    ib = np.broadcast_to(rows_b[:, None, :, None], (B, N_DETS, N_B, K))
    A_a = np.take_along_axis(M_eff, ia, axis=2)[..., :N_A]
    A_b = np.take_along_axis(M_eff, ib, axis=2)[..., :N_B]

    sign_a, lad_a = np.linalg.slogdet(A_a.astype(np.complex128))
    sign_b, lad_b = np.linalg.slogdet(A_b.astype(np.complex128))
    log_dets = np.log(sign_a) + lad_a + np.log(sign_b) + lad_b + log_c[None]

    m = np.max(log_dets.real, axis=1, keepdims=True)
    out = m[:, 0] + np.log(np.sum(np.exp(log_dets - m), axis=1))
    return out.astype(np.complex64)


# revision 26
# speedup vs baseline: 1.1175x; 1.1175x over previous
"""BackflowMLP Trainium2 kernel.

Strategy: the dense MLP (all the FLOPs, dominated by the 512x512 @
512x65536 output matmul) runs on 8 NeuronCores with 2D sharding:
4-way tensor-parallel over the 65536 output dim x 2-way data-parallel
over the batch. TP keeps the PE's moving dim at 256 columns (full
streaming rate); the DP split halves the replicated trunk-MLP work.
Activations are feature-major ([feature_partition, batch_free]) so
biases are per-partition scalars and gelu runs straight off PSUM.
Matmuls run in bf16 (1 cycle/row on TRN2 PE vs 4 for fp32) with fp32
PSUM accumulation; residual adds in fp32. The tiny complex
gather/logdet/logsumexp tail runs on host.
"""

import numpy as np
import ml_dtypes

N_ORB, N_A, N_B, N_DETS = 64, 32, 32, 16
K = 32
H0 = H1 = 512
IN_DIM = 128
OUT_DIM = N_DETS * N_ORB * K            # 32768
OUT2 = 2 * OUT_DIM                      # 65536
B = 512
NCORES = 8
TP, DP = 8, 1
BSH = B // DP                           # 256 batch rows per core
OSH = OUT2 // TP                        # 16384 output features per core
OT = OSH // 128                         # 128 output tiles per core

_CACHE = {}


def _build_nc():
    import concourse.mybir as mybir
    import concourse.tile as tile
    from concourse import bacc

    fp32 = mybir.dt.float32
    bf16 = mybir.dt.bfloat16
    GELU = mybir.ActivationFunctionType.Gelu_apprx_tanh
    INV_SQRT2 = float(1.0 / np.sqrt(2.0))

    nc = bacc.Bacc(
        "TRN2", target_bir_lowering=False, debug=False, num_devices=NCORES
    )

    xT = nc.declare_dram_parameter("xT", [IN_DIM, BSH], bf16, isOutput=False)
    # Wc = Ws0 + gelu(1)*W0, merged on host: s is exactly {0,1}, so
    # gelu(s) = gelu(1)*s and block 0 is a single matmul.
    Wc = nc.declare_dram_parameter("Wc", [IN_DIM, H0], bf16, isOutput=False)
    W1 = nc.declare_dram_parameter("W1", [H0, H1], bf16, isOutput=False)
    Wout = nc.declare_dram_parameter("Wout", [H1, OSH], bf16, isOutput=False)
    bias0 = nc.declare_dram_parameter("bias0", [128, 4], fp32, isOutput=False)
    bias1 = nc.declare_dram_parameter("bias1", [128, 4], fp32, isOutput=False)
    boutp = nc.declare_dram_parameter("boutp", [128, OT], fp32, isOutput=False)
    yT = nc.declare_dram_parameter("yT", [OSH, BSH], bf16, isOutput=True)

    mult = mybir.AluOpType.mult
    add = mybir.AluOpType.add

    with tile.TileContext(nc) as tc:
        with (
            tc.tile_pool(name="wp", bufs=1) as wp,
            tc.tile_pool(name="ap_", bufs=1) as ap_,
            tc.tile_pool(name="op", bufs=8) as op,
            tc.tile_pool(name="pps", bufs=3, space="PSUM") as pps,
        ):
            # ---- persistent loads ----
            xt = wp.tile([128, BSH], bf16, tag="xt")
            nc.sync.dma_start(xt[:], xT[:])
            wc = wp.tile([128, H0], bf16, tag="wc")
            nc.sync.dma_start(wc[:], Wc[:])
            w1 = wp.tile([128, 4 * H1], bf16, tag="w1")
            for kc in range(4):
                nc.sync.dma_start(
                    w1[:, kc * H1:(kc + 1) * H1], W1[kc * 128:(kc + 1) * 128, :]
                )
            b0t = wp.tile([128, 4], fp32, tag="b0t")
            nc.sync.dma_start(b0t[:], bias0[:])
            b1t = wp.tile([128, 4], fp32, tag="b1t")
            nc.sync.dma_start(b1t[:], bias1[:])
            bot = wp.tile([128, OT], fp32, tag="bot")
            nc.sync.dma_start(bot[:], boutp[:])
            # Wout: 4 K-chunks of [128, OSH]; DMA split into 8 pieces along
            # the feature dim so early output tiles can start sooner.
            wout = wp.tile([128, 4 * OSH], bf16, tag="wout")
            NPIECE = 8
            PIECE = OSH // NPIECE
            for piece in range(NPIECE):
                for kc in range(4):
                    nc.sync.dma_start(
                        wout[:, kc * OSH + piece * PIECE: kc * OSH + (piece + 1) * PIECE],
                        Wout[kc * 128:(kc + 1) * 128, piece * PIECE:(piece + 1) * PIECE],
                    )

            # ---- trunk: residual block 0 (skip + gelu branch, merged) ----
            x1 = []
            g1 = []
            for mt in range(4):
                r_ps = pps.tile([128, BSH], fp32, tag="ps_a")
                nc.tensor.matmul(
                    r_ps[:], wc[:, mt * 128:(mt + 1) * 128], xt[:],
                    start=True, stop=True,
                )
                x1t = ap_.tile([128, BSH], fp32, tag=f"x1_{mt}")
                nc.vector.tensor_scalar_add(x1t[:], r_ps[:], b0t[:, mt:mt + 1])
                g1t = ap_.tile([128, BSH], bf16, tag=f"g1_{mt}")
                nc.scalar.activation(g1t[:], x1t[:], GELU)
                x1.append(x1t)
                g1.append(g1t)

            # ---- trunk: residual block 1 (identity skip) ----
            g2 = []
            for mt in range(4):
                h1_ps = pps.tile([128, BSH], fp32, tag="ps_a")
                for kc in range(4):
                    nc.tensor.matmul(
                        h1_ps[:],
                        w1[:, kc * H1 + mt * 128: kc * H1 + (mt + 1) * 128],
                        g1[kc][:],
                        start=(kc == 0), stop=(kc == 3),
                    )
                x2t = ap_.tile([128, BSH], fp32, tag=f"x2_{mt}")
                nc.vector.tensor_add(x2t[:], h1_ps[:], x1[mt][:])
                nc.vector.tensor_scalar_add(x2t[:], x2t[:], b1t[:, mt:mt + 1])
                g2t = ap_.tile([128, BSH], bf16, tag=f"g2_{mt}")
                nc.scalar.activation(g2t[:], x2t[:], GELU)
                g2.append(g2t)

            # ---- big output matmul, OT tiles of [128 feat, BSH batch] ----
            for ot in range(OT):
                y_ps = pps.tile([128, BSH], fp32, tag="ps_y")
                for kc in range(4):
                    nc.tensor.matmul(
                        y_ps[:],
                        wout[:, kc * OSH + ot * 128: kc * OSH + (ot + 1) * 128],
                        g2[kc][:],
                        start=(kc == 0), stop=(kc == 3),
                    )
                # ACT does the PSUM read + scale (canonical Copy epilogue),
                # DVE only a fast bf16 SBUF bias-add: neither paces the PE
                yt2 = op.tile([128, BSH], bf16, tag="yt2")
                nc.scalar.activation(
                    yt2[:], y_ps[:], mybir.ActivationFunctionType.Copy,
                    scale=INV_SQRT2,
                )
                yt = op.tile([128, BSH], bf16, tag="yt")
                nc.vector.tensor_scalar_add(yt[:], yt2[:], bot[:, ot:ot + 1])
                nc.sync.dma_start(yT[ot * 128:(ot + 1) * 128, :], yt[:])
    nc.compile()
    return nc


def _get_nc():
    if "nc" not in _CACHE:
        _CACHE["nc"] = _build_nc()
    return _CACHE["nc"]


def kernel(**inputs):
    import os
    import time
    os.environ["BASS_NEVER_TRACE"] = "1"   # NTFF hook module absent in this build
    from concourse import bass_utils

    s = np.asarray(inputs["s"])
    W0 = np.asarray(inputs["W0"], np.float32)
    b0 = np.asarray(inputs["b0"], np.float32)
    Ws0 = np.asarray(inputs["Ws0"], np.float32)
    bs0 = np.asarray(inputs["bs0"], np.float32)
    W1 = np.asarray(inputs["W1"], np.float32)
    b1 = np.asarray(inputs["b1"], np.float32)
    Wout = np.asarray(inputs["Wout"], np.float32)
    bout = np.asarray(inputs["bout"], np.float32)
    M = np.asarray(inputs["M"])
    log_c = np.asarray(inputs["log_c"])

    bf = ml_dtypes.bfloat16
    xT_full = np.ascontiguousarray(s.astype(np.float32).T).astype(bf)   # [128, 512]
    bias0 = np.ascontiguousarray((b0 + bs0).reshape(4, 128).T).astype(np.float32)
    bias1 = np.ascontiguousarray(b1.reshape(4, 128).T).astype(np.float32)
    # s is {0,1} exactly, so gelu(s) = gelu(1)*s: fold the gelu branch of
    # block 0 into the skip projection (tanh-approx gelu at x=1, fp64).
    g1c = 0.5 * (1.0 + np.tanh(np.sqrt(2.0 / np.pi) * (1.0 + 0.044715)))
    Wcb = (Ws0.astype(np.float64) + g1c * W0.astype(np.float64)).astype(bf)
    W1b = W1.astype(bf)

    wsh = []
    bsh = []
    for tp in range(TP):
        wsh.append(np.ascontiguousarray(Wout[:, tp * OSH:(tp + 1) * OSH]).astype(bf))
        bo = bout[tp * OSH:(tp + 1) * OSH] / np.sqrt(2.0)
        bsh.append(np.ascontiguousarray(bo.reshape(OT, 128).T).astype(np.float32))

    in_maps = []
    for i in range(NCORES):
        dp, tp = divmod(i, TP)
        in_maps.append({
            "xT": np.ascontiguousarray(xT_full[:, dp * BSH:(dp + 1) * BSH]),
            "Wc": Wcb,
            "W1": W1b,
            "Wout": wsh[tp],
            "bias0": bias0,
            "bias1": bias1,
            "boutp": bsh[tp],
        })

    nc = _get_nc()
    t0 = time.perf_counter()
    res = bass_utils.run_bass_kernel_spmd(nc, in_maps, core_ids=list(range(NCORES)))
    _CACHE["last_exec_ns"] = res.exec_time_ns
    _CACHE["last_wall_ns"] = (time.perf_counter() - t0) * 1e9

    y = np.empty((B, OUT2), np.float32)
    for i in range(NCORES):
        dp, tp = divmod(i, TP)
        y[dp * BSH:(dp + 1) * BSH, tp * OSH:(tp + 1) * OSH] = (
            res.results[i]["yT"].astype(np.float32).T
        )

    # ---- host tail: complex assembly, gather, logdet, logsumexp ----
    re, im = y[:, :OUT_DIM], y[:, OUT_DIM:]
    delta = (re + 1j * im).astype(np.complex64).reshape(B, N_DETS, N_ORB, K)
    M_eff = M[None].astype(np.complex64) + delta

    rows_a = np.argsort(1 - s[:, :N_ORB], axis=-1, kind="stable")[:, :N_A]
    rows_b = np.argsort(1 - s[:, N_ORB:], axis=-1, kind="stable")[:, :N_B]
    ia = np.broadcast_to(rows_a[:, None, :, None], (B, N_DETS, N_A, K))
    ib = np.broadcast_to(rows_b[:, None, :, None], (B, N_DETS, N_B, K))
    A_a = np.take_along_axis(M_eff, ia, axis=2)[..., :N_A]
    A_b = np.take_along_axis(M_eff, ib, axis=2)[..., :N_B]

    sign_a, lad_a = np.linalg.slogdet(A_a.astype(np.complex128))
    sign_b, lad_b = np.linalg.slogdet(A_b.astype(np.complex128))
    log_dets = np.log(sign_a) + lad_a + np.log(sign_b) + lad_b + log_c[None]

    m = np.max(log_dets.real, axis=1, keepdims=True)
    out = m[:, 0] + np.log(np.sum(np.exp(log_dets - m), axis=1))
    return out.astype(np.complex64)
